# revision 6
# baseline (speedup 1.0000x reference)
"""Trainium2 Bass kernel for DepthFFN (histogram_binning).

Computes, for inputs
  image_features  (2, 32, 47, 156)  f32
  depth_logits    (2, 121, 47, 156) f32
  depth_maps      (2, 376, 1248)    f32
  depth_target_bin(2, 47, 156)      i32
the reference outputs
  frustum_features        (2, 32, 120, 47, 156) = softmax(logits)[:, :120] x image_features
  frustum_features_target (2, 32, 120, 47, 156) = onehot(bin)[:, :120]    x image_features
  pooled_depth            (2, 47, 156)          = sparse 8x8 avg pool of depth_maps

Sharding: 8 cores = (batch b in {0,1}) x (depth chunk dc in {0..3}, 30 bins
each). Each core writes its [30, 32, 7332] d-major slice of both frustum
tensors, and redundantly computes the (tiny) pooled output; the host takes
core 0's copy.

The kernel is column-pipelined: the hw = 47*156 = 7332 free axis is cut into
4 sections (3 x 2048 + 1188); every step of the computation is column-local,
so section s+1's prologue overlaps section s's main loop and output DMA.

Per section:
  prologue: exp = Exp(logits) (ACT); partition-sum via ones-matmul (PE);
    exact reciprocal via a DRAM-bounce reshape to ~64 lanes (DVE); inverse
    broadcast over partitions via ones-matmul (PE); probs = exp * inv (DVE),
    then split probs into bf16 hi + bf16 lo (DVE) so the main-loop
    replication matmuls run at bf16 rate (2 cycles/col instead of fp32's 4)
    while keeping ~fp32 precision (hi+lo accumulate in fp32 PSUM);
    img_rep [128, w] (channel p%32, fp32 selection matmul + ACT copyback);
    bin_rep [128, w] (bf16 ones-matmul broadcast of the integer bin row —
    exact — + ACT copyback).
  main: for each of 8 d-tiles (4 depth bins x 32 channels on partitions,
    last tile 2x32): selection-replicate probs hi+lo into PSUM (PE),
    multiply with img_rep (DVE) -> frustum tile; fused
    (bin_rep == d(p)) * img_rep scalar_tensor_tensor on GpSimd -> target
    tile; 1 MB-class DMA per tile/section to DRAM.
"""

import numpy as np
import ml_dtypes

import concourse.bacc as bacc
import concourse.bass as bass
import concourse.mybir as mybir
import concourse.tile as tile
from concourse.bass_utils import run_bass_kernel_spmd

F32 = mybir.dt.float32
BF16 = mybir.dt.bfloat16
AF = mybir.ActivationFunctionType
OP = mybir.AluOpType
AX = mybir.AxisListType

B, C, D, DP1 = 2, 32, 120, 121
H, W = 47, 156
HW = H * W  # 7332
ND = 30  # depth bins per core
NCORES = 8
CW = 512  # matmul moving-operand chunk (one fp32 PSUM bank)
SECW = 2048  # pipeline section width (one PSUM tile, 4 banks)
# sections: [0,2048) [2048,4096) [4096,6144) [6144,7332)
SECS = [(s, min(SECW, HW - s)) for s in range(0, HW, SECW)]
# per-section reshape factors for the reciprocal bounce (rows x cols = w)
RECIP_SHAPE = {2048: (64, 32), 1188: (44, 27)}
NT = 8  # d-tiles per core: 7 x (4 bins x 32 ch) + 1 x (2 bins x 32 ch)
PQ = 312  # pooling quarter width (156*8/4 columns of depth_maps)


def build_program():
    nc = bacc.Bacc(
        "TRN2",
        target_bir_lowering=False,
        debug=False,
        num_devices=NCORES,
    )

    img_d = nc.dram_tensor("img", [C, HW], F32, kind="ExternalInput").ap()
    logits_d = nc.dram_tensor("logits", [DP1, HW], F32, kind="ExternalInput").ap()
    binf16_d = nc.dram_tensor("binf16", [1, HW], BF16, kind="ExternalInput").ap()
    dvals_d = nc.dram_tensor("dvals", [128, NT], F32, kind="ExternalInput").ap()
    sel_d = nc.dram_tensor("sel", [DP1, ND * C], BF16, kind="ExternalInput").ap()
    sel32_d = nc.dram_tensor("sel32", [C, 128], F32, kind="ExternalInput").ap()
    onescol_d = nc.dram_tensor("onescol", [DP1, 1], F32, kind="ExternalInput").ap()
    onesrow_d = nc.dram_tensor("onesrow", [1, 128], F32, kind="ExternalInput").ap()
    onesrow16_d = nc.dram_tensor(
        "onesrow16", [1, 128], BF16, kind="ExternalInput"
    ).ap()
    dmaps_d = nc.dram_tensor("dmaps", [94, 8, 1248], F32, kind="ExternalInput").ap()

    out_f_d = nc.dram_tensor("out_f", [ND * C, HW], F32, kind="ExternalOutput").ap()
    out_t_d = nc.dram_tensor("out_t", [ND * C, HW], F32, kind="ExternalOutput").ap()
    pooled_d = nc.dram_tensor("pooled", [94, 156], F32, kind="ExternalOutput").ap()

    with tile.TileContext(nc) as tc:
        with (
            tc.tile_pool(name="const", bufs=1) as constp,
            tc.tile_pool(name="sec", bufs=2) as secp,
            tc.tile_pool(name="outf", bufs=2) as outfp,
            tc.tile_pool(name="outt", bufs=2) as outtp,
            tc.tile_pool(name="psum", bufs=2, space="PSUM") as psp,
            tc.tile_pool(name="dram", bufs=1, space="DRAM") as dramp,
            tc.tile_pool(name="poolx", bufs=1) as poolxp,
        ):
            # ---- constants ----
            sel_s = constp.tile([DP1, ND * C], BF16, tag="sel")
            nc.sync.dma_start(sel_s, sel_d)
            sel32_s = constp.tile([C, 128], F32, tag="sel32")
            nc.sync.dma_start(sel32_s, sel32_d)
            onescol_s = constp.tile([DP1, 1], F32, tag="onescol")
            nc.sync.dma_start(onescol_s, onescol_d)
            onesrow_s = constp.tile([1, 128], F32, tag="onesrow")
            nc.sync.dma_start(onesrow_s, onesrow_d)
            onesrow16_s = constp.tile([1, 128], BF16, tag="onesrow16")
            nc.sync.dma_start(onesrow16_s, onesrow16_d)
            dvals_s = constp.tile([128, NT], F32, tag="dvals")
            nc.sync.dma_start(dvals_s, dvals_d)

            sums_b = dramp.tile([1, HW], F32, tag="sums")
            inv_b = dramp.tile([1, HW], F32, tag="inv")

            for c0, w in SECS:
                cq = [(q, min(CW, w - q)) for q in range(0, w, CW)]
                sec = slice(c0, c0 + w)

                # ---- section prologue ----
                exp_s = secp.tile([DP1, SECW], F32, tag="exp")
                nc.sync.dma_start(exp_s[:, :w], logits_d[:, sec])
                nc.scalar.activation(exp_s[:, :w], exp_s[:, :w], AF.Exp)

                img_s = secp.tile([C, SECW], F32, tag="imgsec")
                nc.sync.dma_start(img_s[:, :w], img_d[:, sec])
                bin_s = secp.tile([1, SECW], BF16, tag="binsec")
                nc.sync.dma_start(bin_s[:, :w], binf16_d[:, sec])

                # partition-sum of exp -> DRAM bounce -> reciprocal -> back
                ps = psp.tile([128, SECW], F32, tag="ps")
                for q, qw in cq:
                    nc.tensor.matmul(
                        ps[:1, q : q + qw],
                        onescol_s,
                        exp_s[:, q : q + qw],
                        start=True,
                        stop=True,
                    )
                sum_s = secp.tile([1, SECW], F32, tag="sumsec")
                nc.scalar.copy(sum_s[:, :w], ps[:1, :w])
                nc.sync.dma_start(sums_b[:, sec], sum_s[:, :w])

                rr, rc = RECIP_SHAPE[w]
                r_s = secp.tile([rr, rc], F32, tag="rsec")
                nc.sync.dma_start(
                    r_s, sums_b[:, sec].rearrange("o (p q) -> (o p) q", q=rc)
                )
                nc.vector.reciprocal(r_s, r_s)
                nc.sync.dma_start(
                    inv_b[:, sec].rearrange("o (p q) -> (o p) q", q=rc), r_s
                )
                inv_s = secp.tile([1, SECW], F32, tag="invsec")
                nc.sync.dma_start(inv_s[:, :w], inv_b[:, sec])

                # probs = exp * (1/sum); split into bf16 hi + lo
                psi = psp.tile([128, SECW], F32, tag="ps")
                for q, qw in cq:
                    nc.tensor.matmul(
                        psi[:DP1, q : q + qw],
                        onesrow_s[:1, :DP1],
                        inv_s[:1, q : q + qw],
                        start=True,
                        stop=True,
                    )
                nc.vector.tensor_tensor(
                    exp_s[:, :w], exp_s[:, :w], psi[:DP1, :w], OP.mult
                )
                phi_s = secp.tile([DP1, SECW], BF16, tag="phi")
                nc.vector.tensor_copy(phi_s[:, :w], exp_s[:, :w])
                plo_s = secp.tile([DP1, SECW], BF16, tag="plo")
                nc.vector.tensor_tensor(
                    plo_s[:, :w], exp_s[:, :w], phi_s[:, :w], OP.subtract
                )

                # img_rep: partition p <- img[p % 32]
                psm = psp.tile([128, SECW], F32, tag="ps")
                for q, qw in cq:
                    nc.tensor.matmul(
                        psm[:, q : q + qw],
                        sel32_s,
                        img_s[:, q : q + qw],
                        start=True,
                        stop=True,
                    )
                irep_s = secp.tile([128, SECW], F32, tag="irep")
                nc.scalar.copy(irep_s[:, :w], psm[:, :w])

                # bin_rep: integer bin row broadcast to 128 partitions (bf16
                # ones-matmul is exact for integers <= 256)
                psb = psp.tile([128, SECW], F32, tag="ps")
                for q, qw in cq:
                    nc.tensor.matmul(
                        psb[:, q : q + qw],
                        onesrow16_s,
                        bin_s[:1, q : q + qw],
                        start=True,
                        stop=True,
                    )
                brep_s = secp.tile([128, SECW], F32, tag="brep")
                nc.scalar.copy(brep_s[:, :w], psb[:, :w])

                # ---- main loop over d-tiles ----
                for t in range(NT):
                    pt = 128 if t < NT - 1 else 64
                    m0 = 128 * t
                    psf = psp.tile([128, SECW], F32, tag="ps")
                    for q, qw in cq:
                        nc.tensor.matmul(
                            psf[:pt, q : q + qw],
                            sel_s[:, m0 : m0 + pt],
                            phi_s[:, q : q + qw],
                            start=True,
                            stop=False,
                        )
                        nc.tensor.matmul(
                            psf[:pt, q : q + qw],
                            sel_s[:, m0 : m0 + pt],
                            plo_s[:, q : q + qw],
                            start=False,
                            stop=True,
                        )
                    of = outfp.tile([128, SECW], F32, tag="of")
                    nc.vector.tensor_tensor(
                        of[:pt, :w], psf[:pt, :w], irep_s[:pt, :w], OP.mult
                    )
                    nc.sync.dma_start(out_f_d[m0 : m0 + pt, sec], of[:pt, :w])

                    # target: DVE builds the one-hot mask (tensor_scalar runs
                    # in 2x mode), GpSimd does the 2-input multiply so the DVE
                    # stays free for the frustum stream
                    mk2 = outtp.tile([128, SECW], F32, tag="mask")
                    nc.vector.tensor_scalar(
                        mk2[:pt, :w],
                        brep_s[:pt, :w],
                        dvals_s[:pt, t : t + 1],
                        None,
                        OP.is_equal,
                    )
                    ot = outtp.tile([128, SECW], F32, tag="ot")
                    nc.gpsimd.tensor_tensor(
                        ot[:pt, :w], mk2[:pt, :w], irep_s[:pt, :w], OP.mult
                    )
                    nc.sync.dma_start(out_t_d[m0 : m0 + pt, sec], ot[:pt, :w])

            # ---- sparse average pooling (both batches; 4 column quarters) ----
            pooled_s = poolxp.tile([94, 156], F32, tag="pooled")
            for quarter in range(4):
                h0 = quarter * PQ
                xp = poolxp.tile([94, 8, PQ], F32, tag="xp", bufs=1)
                nc.sync.dma_start(xp, dmaps_d[:, :, h0 : h0 + PQ])
                mk = poolxp.tile([94, 8, PQ], F32, tag="mk", bufs=1)
                # depth values are >= 0, so Sign(x) == (x != 0)
                nc.scalar.activation(mk, xp, AF.Sign)
                sv = poolxp.tile([94, PQ // 8], F32, tag="sv", bufs=2)
                sm = poolxp.tile([94, PQ // 8], F32, tag="sm", bufs=2)
                nc.vector.tensor_reduce(
                    sv,
                    xp.rearrange("p r (j q) -> p j r q", q=8),
                    axis=AX.XY,
                    op=OP.add,
                )
                nc.vector.tensor_reduce(
                    sm,
                    mk.rearrange("p r (j q) -> p j r q", q=8),
                    axis=AX.XY,
                    op=OP.add,
                )
                # ref: (sum/64) / (cnt/64 + 1e-10)
                nc.vector.tensor_scalar(
                    sm, sm, 1.0 / 64.0, 1e-10, OP.mult, OP.add
                )
                nc.vector.reciprocal(sm, sm)
                nc.vector.scalar_tensor_tensor(
                    pooled_s[:, quarter * (PQ // 8) : (quarter + 1) * (PQ // 8)],
                    sv,
                    1.0 / 64.0,
                    sm,
                    OP.mult,
                    OP.mult,
                )
            nc.sync.dma_start(pooled_d, pooled_s)

    nc.finalize()
    return nc


_CACHE: dict = {}


def _get_program():
    if "nc" not in _CACHE:
        _CACHE["nc"] = build_program()
    return _CACHE["nc"]


def _make_in_maps(image_features, depth_logits, depth_maps, depth_target_bin):
    img = np.ascontiguousarray(np.asarray(image_features, np.float32)).reshape(
        B, C, HW
    )
    logits = np.ascontiguousarray(np.asarray(depth_logits, np.float32)).reshape(
        B, DP1, HW
    )
    binf16 = (
        np.asarray(depth_target_bin)
        .astype(np.float32)
        .reshape(B, 1, HW)
        .astype(ml_dtypes.bfloat16)
    )
    dmaps = np.ascontiguousarray(np.asarray(depth_maps, np.float32)).reshape(
        94, 8, 1248
    )

    sel32 = np.zeros((C, 128), np.float32)
    sel32[np.arange(128) % C, np.arange(128)] = 1.0
    onescol = np.ones((DP1, 1), np.float32)
    onesrow = np.ones((1, 128), np.float32)
    onesrow16 = np.ones((1, 128), ml_dtypes.bfloat16)

    in_maps = []
    for core in range(NCORES):
        b, dc = divmod(core, 4)
        d0 = ND * dc
        # selection matrix: column m of d-tile t selects depth row d0+4t+m//32
        sel = np.zeros((DP1, ND * C), np.float32)
        dvals = np.full((128, NT), -7.0, np.float32)
        for t in range(NT):
            pt = 128 if t < NT - 1 else 64
            for m in range(pt):
                k = d0 + 4 * t + m // 32
                sel[k, 128 * t + m] = 1.0
            dvals[:pt, t] = d0 + 4 * t + np.arange(pt) // 32
        in_maps.append(
            {
                "img": img[b],
                "logits": logits[b],
                "binf16": binf16[b],
                "dvals": dvals,
                "sel": sel.astype(ml_dtypes.bfloat16),
                "sel32": sel32,
                "onescol": onescol,
                "onesrow": onesrow,
                "onesrow16": onesrow16,
                "dmaps": dmaps,
            }
        )
    return in_maps


def kernel(
    image_features,
    depth_logits,
    depth_maps,
    depth_target_bin,
    _trace=False,
    _tmpdir=None,
):
    nc = _get_program()
    in_maps = _make_in_maps(
        image_features, depth_logits, depth_maps, depth_target_bin
    )
    res = run_bass_kernel_spmd(
        nc,
        in_maps,
        core_ids=list(range(NCORES)),
        trace=_trace,
        tmpdir=_tmpdir,
    )
    _CACHE["last_results"] = res

    frustum = np.empty((B, C, D, H, W), np.float32)
    frustum_t = np.empty((B, C, D, H, W), np.float32)
    for core in range(NCORES):
        b, dc = divmod(core, 4)
        r = res.results[core]
        f = r["out_f"].reshape(ND, C, H, W).transpose(1, 0, 2, 3)
        ft = r["out_t"].reshape(ND, C, H, W).transpose(1, 0, 2, 3)
        frustum[b, :, ND * dc : ND * (dc + 1)] = f
        frustum_t[b, :, ND * dc : ND * (dc + 1)] = ft
    pooled = res.results[0]["pooled"].reshape(B, H, W).copy()
    return frustum, frustum_t, pooled


# revision 7
# speedup vs baseline: 1.0990x; 1.0990x over previous
"""Trainium2 Bass kernel for DepthFFN (histogram_binning).

Computes, for inputs
  image_features  (2, 32, 47, 156)  f32
  depth_logits    (2, 121, 47, 156) f32
  depth_maps      (2, 376, 1248)    f32
  depth_target_bin(2, 47, 156)      i32
the reference outputs
  frustum_features        (2, 32, 120, 47, 156) = softmax(logits)[:, :120] x image_features
  frustum_features_target (2, 32, 120, 47, 156) = onehot(bin)[:, :120]    x image_features
  pooled_depth            (2, 47, 156)          = sparse 8x8 avg pool of depth_maps

Sharding: 8 cores = (batch b in {0,1}) x (depth chunk dc in {0..3}, 30 bins
each). Each core writes its [30, 32, 7332] d-major slice of both frustum
tensors, and redundantly computes the (tiny) pooled output; the host takes
core 0's copy.

The kernel is column-pipelined: the hw = 47*156 = 7332 free axis is cut into
4 sections (3 x 2048 + 1188); every step is column-local, so sections overlap
through the Tile scheduler. Partition-broadcasts are done with DMA (step-0 /
replicated access patterns), keeping the PE for the only two real matmul
jobs: the softmax partition-sum and the per-d-tile probs replication.

Per section:
  - one pooling quarter (DVE/ACT filler work, independent of everything);
  - exp = Exp(logits) in place (ACT); partition-sum via ones-matmul (PE,
    fp32); exact reciprocal via a DRAM-bounce reshape to ~64 lanes (DVE);
    1/sum broadcast back over 121 partitions by a replicating DMA read of
    the bounce row; probs = exp * inv (DVE), split into bf16 hi + lo (DVE)
    so the main-loop replication matmuls run at bf16 rate with ~fp32
    precision (hi+lo accumulate into the same fp32 PSUM bank);
  - img_rep [128, w] = image rows replicated 4x (SBUF->SBUF DMA);
    bin_rep [128, w] bf16 = integer bin row replicated 128x (DMA from DRAM);
  - main loop over 8 d-tiles (4 depth bins x 32 channels on partitions,
    last tile 2x32): selection-matmul probs hi+lo into PSUM (PE, bf16,
    exact 0/1 weights), multiply with img_rep (DVE) -> frustum tile;
    one-hot mask = (bin_rep == d(p)) via tensor_scalar (DVE, bf16 4x mode),
    mask * img_rep on GpSimd -> target tile; ~1 MB DMA stores.
"""

import numpy as np
import ml_dtypes

import concourse.bacc as bacc
import concourse.bass as bass
import concourse.mybir as mybir
import concourse.tile as tile
from concourse.bass_utils import run_bass_kernel_spmd

F32 = mybir.dt.float32
BF16 = mybir.dt.bfloat16
AF = mybir.ActivationFunctionType
OP = mybir.AluOpType
AX = mybir.AxisListType

B, C, D, DP1 = 2, 32, 120, 121
H, W = 47, 156
HW = H * W  # 7332
ND = 30  # depth bins per core
NCORES = 8
CW = 512  # matmul moving-operand chunk (one fp32 PSUM bank)
PSW = 1024  # PSUM tile width (2 banks)
SECW = 2048  # pipeline section width
SECS = [(s, min(SECW, HW - s)) for s in range(0, HW, SECW)]
# per-section reshape factors for the reciprocal bounce (rows x cols = w)
RECIP_SHAPE = {2048: (64, 32), 1188: (44, 27)}
NT = 8  # d-tiles per core: 7 x (4 bins x 32 ch) + 1 x (2 bins x 32 ch)
PQ = 312  # pooling quarter width (in depth_maps columns)


def build_program():
    nc = bacc.Bacc(
        "TRN2",
        target_bir_lowering=False,
        debug=False,
        num_devices=NCORES,
    )

    img_d = nc.dram_tensor("img", [C, HW], F32, kind="ExternalInput").ap()
    logits_d = nc.dram_tensor("logits", [DP1, HW], F32, kind="ExternalInput").ap()
    binf16_d = nc.dram_tensor("binf16", [1, HW], BF16, kind="ExternalInput").ap()
    dvals_d = nc.dram_tensor("dvals", [128, NT], F32, kind="ExternalInput").ap()
    sel_d = nc.dram_tensor("sel", [DP1, ND * C], BF16, kind="ExternalInput").ap()
    onescol_d = nc.dram_tensor("onescol", [DP1, 1], F32, kind="ExternalInput").ap()
    dmaps_d = nc.dram_tensor("dmaps", [94, 8, 1248], F32, kind="ExternalInput").ap()

    out_f_d = nc.dram_tensor("out_f", [ND * C, HW], F32, kind="ExternalOutput").ap()
    out_t_d = nc.dram_tensor("out_t", [ND * C, HW], F32, kind="ExternalOutput").ap()
    pooled_d = nc.dram_tensor("pooled", [94, 156], F32, kind="ExternalOutput").ap()

    with tile.TileContext(nc) as tc:
        with (
            tc.tile_pool(name="const", bufs=1) as constp,
            tc.tile_pool(name="sec", bufs=2) as secp,
            tc.tile_pool(name="outf", bufs=3) as outfp,
            tc.tile_pool(name="outt", bufs=3) as outtp,
            tc.tile_pool(name="psum", bufs=4, space="PSUM") as psp,
            tc.tile_pool(name="dram", bufs=1, space="DRAM") as dramp,
            tc.tile_pool(name="poolx", bufs=1) as poolxp,
        ):
            # ---- constants ----
            sel_s = constp.tile([DP1, ND * C], BF16, tag="sel")
            nc.sync.dma_start(sel_s, sel_d)
            onescol_s = constp.tile([DP1, 1], F32, tag="onescol")
            nc.sync.dma_start(onescol_s, onescol_d)
            dvals_s = constp.tile([128, NT], F32, tag="dvals")
            nc.sync.dma_start(dvals_s, dvals_d)

            sums_b = dramp.tile([1, HW], F32, tag="sums")
            inv_b = dramp.tile([1, HW], F32, tag="inv")

            pooled_s = poolxp.tile([94, 156], F32, tag="pooled")

            for si, (c0, w) in enumerate(SECS):
                sec = slice(c0, c0 + w)
                # psum tiles in this section: [(start, width), ...]
                pq = [(q, min(PSW, w - q)) for q in range(0, w, PSW)]

                # ---- pooling quarter (independent filler work) ----
                h0 = si * PQ
                xp = poolxp.tile([94, 8, PQ], F32, tag="xp", bufs=2)
                nc.sync.dma_start(xp, dmaps_d[:, :, h0 : h0 + PQ])
                mk = poolxp.tile([94, 8, PQ], F32, tag="mk", bufs=2)
                # depth values are >= 0, so Sign(x) == (x != 0)
                nc.scalar.activation(mk, xp, AF.Sign)
                sv = poolxp.tile([94, PQ // 8], F32, tag="sv", bufs=2)
                sm = poolxp.tile([94, PQ // 8], F32, tag="sm", bufs=2)
                nc.vector.tensor_reduce(
                    sv,
                    xp.rearrange("p r (j q) -> p j r q", q=8),
                    axis=AX.XY,
                    op=OP.add,
                )
                nc.vector.tensor_reduce(
                    sm,
                    mk.rearrange("p r (j q) -> p j r q", q=8),
                    axis=AX.XY,
                    op=OP.add,
                )
                # ref: (sum/64) / (cnt/64 + 1e-10)
                nc.vector.tensor_scalar(
                    sm, sm, 1.0 / 64.0, 1e-10, OP.mult, OP.add
                )
                nc.vector.reciprocal(sm, sm)
                nc.vector.scalar_tensor_tensor(
                    pooled_s[:, si * (PQ // 8) : (si + 1) * (PQ // 8)],
                    sv,
                    1.0 / 64.0,
                    sm,
                    OP.mult,
                    OP.mult,
                )

                # ---- softmax prologue ----
                exp_s = secp.tile([DP1, SECW], F32, tag="exp")
                nc.sync.dma_start(exp_s[:, :w], logits_d[:, sec])
                nc.scalar.activation(exp_s[:, :w], exp_s[:, :w], AF.Exp)

                img_s = secp.tile([C, SECW], F32, tag="imgsec")
                nc.sync.dma_start(img_s[:, :w], img_d[:, sec])

                # partition-sum of exp -> DRAM bounce -> reciprocal -> back
                sum_s = secp.tile([1, SECW], F32, tag="sumsec")
                for q, qw in pq:
                    ps = psp.tile([128, PSW], F32, tag="ps")
                    for k in range(0, qw, CW):
                        kw = min(CW, qw - k)
                        nc.tensor.matmul(
                            ps[:1, k : k + kw],
                            onescol_s,
                            exp_s[:, q + k : q + k + kw],
                            start=True,
                            stop=True,
                        )
                    nc.scalar.copy(sum_s[:, q : q + qw], ps[:1, :qw])
                nc.sync.dma_start(sums_b[:, sec], sum_s[:, :w])

                rr, rc = RECIP_SHAPE[w]
                r_s = secp.tile([rr, rc], F32, tag="rsec")
                nc.sync.dma_start(
                    r_s, sums_b[:, sec].rearrange("o (p q) -> (o p) q", q=rc)
                )
                nc.vector.reciprocal(r_s, r_s)
                nc.sync.dma_start(
                    inv_b[:, sec].rearrange("o (p q) -> (o p) q", q=rc), r_s
                )
                # 1/sum broadcast over 121 partitions: replicating DMA read
                invrep_s = secp.tile([DP1, SECW], F32, tag="invrep")
                nc.gpsimd.dma_start(
                    invrep_s[:, :w],
                    bass.AP(inv_b.tensor, inv_b.offset + c0, [[0, DP1], [1, w]]),
                )

                # probs = exp * (1/sum); split into bf16 hi + lo
                nc.vector.tensor_tensor(
                    exp_s[:, :w], exp_s[:, :w], invrep_s[:, :w], OP.mult
                )
                phi_s = secp.tile([DP1, SECW], BF16, tag="phi")
                nc.vector.tensor_copy(phi_s[:, :w], exp_s[:, :w])
                plo_s = secp.tile([DP1, SECW], BF16, tag="plo")
                nc.vector.tensor_tensor(
                    plo_s[:, :w], exp_s[:, :w], phi_s[:, :w], OP.subtract
                )

                # img_rep: partition p <- img[p % 32] (4 SBUF->SBUF copies)
                irep_s = secp.tile([128, SECW], F32, tag="irep")
                for k in range(4):
                    nc.sync.dma_start(
                        irep_s[32 * k : 32 * (k + 1), :w], img_s[:, :w]
                    )
                # bin_rep: integer bin row replicated to 128 partitions (bf16)
                brep_s = secp.tile([128, SECW], BF16, tag="brep")
                nc.gpsimd.dma_start(
                    brep_s[:, :w],
                    bass.AP(
                        binf16_d.tensor, binf16_d.offset + c0, [[0, 128], [1, w]]
                    ),
                )

                # ---- main loop over d-tiles ----
                for t in range(NT):
                    pt = 128 if t < NT - 1 else 64
                    m0 = 128 * t
                    of = outfp.tile([128, SECW], F32, tag="of")
                    for q, qw in pq:
                        psf = psp.tile([128, PSW], F32, tag="ps")
                        for k in range(0, qw, CW):
                            kw = min(CW, qw - k)
                            nc.tensor.matmul(
                                psf[:pt, k : k + kw],
                                sel_s[:, m0 : m0 + pt],
                                phi_s[:, q + k : q + k + kw],
                                start=True,
                                stop=False,
                            )
                            nc.tensor.matmul(
                                psf[:pt, k : k + kw],
                                sel_s[:, m0 : m0 + pt],
                                plo_s[:, q + k : q + k + kw],
                                start=False,
                                stop=True,
                            )
                        nc.vector.tensor_tensor(
                            of[:pt, q : q + qw],
                            psf[:pt, :qw],
                            irep_s[:pt, q : q + qw],
                            OP.mult,
                        )
                    nc.sync.dma_start(out_f_d[m0 : m0 + pt, sec], of[:pt, :w])

                    # target: DVE builds the one-hot mask in bf16 (4x mode),
                    # GpSimd does the 2-input multiply
                    mk2 = outtp.tile([128, SECW], BF16, tag="mask")
                    nc.vector.tensor_scalar(
                        mk2[:pt, :w],
                        brep_s[:pt, :w],
                        dvals_s[:pt, t : t + 1],
                        None,
                        OP.is_equal,
                    )
                    ot = outtp.tile([128, SECW], F32, tag="ot")
                    nc.gpsimd.tensor_tensor(
                        ot[:pt, :w], mk2[:pt, :w], irep_s[:pt, :w], OP.mult
                    )
                    nc.sync.dma_start(out_t_d[m0 : m0 + pt, sec], ot[:pt, :w])

            nc.sync.dma_start(pooled_d, pooled_s)

    nc.finalize()
    return nc


_CACHE: dict = {}


def _get_program():
    if "nc" not in _CACHE:
        _CACHE["nc"] = build_program()
    return _CACHE["nc"]


def _make_in_maps(image_features, depth_logits, depth_maps, depth_target_bin):
    img = np.ascontiguousarray(np.asarray(image_features, np.float32)).reshape(
        B, C, HW
    )
    logits = np.ascontiguousarray(np.asarray(depth_logits, np.float32)).reshape(
        B, DP1, HW
    )
    binf16 = (
        np.asarray(depth_target_bin)
        .astype(np.float32)
        .reshape(B, 1, HW)
        .astype(ml_dtypes.bfloat16)
    )
    dmaps = np.ascontiguousarray(np.asarray(depth_maps, np.float32)).reshape(
        94, 8, 1248
    )

    onescol = np.ones((DP1, 1), np.float32)

    in_maps = []
    for core in range(NCORES):
        b, dc = divmod(core, 4)
        d0 = ND * dc
        # selection matrix: column m of d-tile t selects depth row d0+4t+m//32
        sel = np.zeros((DP1, ND * C), np.float32)
        dvals = np.full((128, NT), -7.0, np.float32)
        for t in range(NT):
            pt = 128 if t < NT - 1 else 64
            for m in range(pt):
                k = d0 + 4 * t + m // 32
                sel[k, 128 * t + m] = 1.0
            dvals[:pt, t] = d0 + 4 * t + np.arange(pt) // 32
        in_maps.append(
            {
                "img": img[b],
                "logits": logits[b],
                "binf16": binf16[b],
                "dvals": dvals,
                "sel": sel.astype(ml_dtypes.bfloat16),
                "onescol": onescol,
                "dmaps": dmaps,
            }
        )
    return in_maps


def kernel(
    image_features,
    depth_logits,
    depth_maps,
    depth_target_bin,
    _trace=False,
    _tmpdir=None,
):
    nc = _get_program()
    in_maps = _make_in_maps(
        image_features, depth_logits, depth_maps, depth_target_bin
    )
    res = run_bass_kernel_spmd(
        nc,
        in_maps,
        core_ids=list(range(NCORES)),
        trace=_trace,
        tmpdir=_tmpdir,
    )
    _CACHE["last_results"] = res

    frustum = np.empty((B, C, D, H, W), np.float32)
    frustum_t = np.empty((B, C, D, H, W), np.float32)
    for core in range(NCORES):
        b, dc = divmod(core, 4)
        r = res.results[core]
        f = r["out_f"].reshape(ND, C, H, W).transpose(1, 0, 2, 3)
        ft = r["out_t"].reshape(ND, C, H, W).transpose(1, 0, 2, 3)
        frustum[b, :, ND * dc : ND * (dc + 1)] = f
        frustum_t[b, :, ND * dc : ND * (dc + 1)] = ft
    pooled = res.results[0]["pooled"].reshape(B, H, W).copy()
    return frustum, frustum_t, pooled
